# revision 10
# baseline (speedup 1.0000x reference)
"""BitfieldLinear (vq_codebook) Trainium2 kernel, fp8-decomposed.

y = x @ W^T + bias with W = r[:,None]*basis[idx] + scales[:,None]*(q-128)/127.

Instead of materializing W (the baseline), decompose:
    y[n,o] = Gg[o,n] + s'_o * R[o,n] + bias_o
      G  [b,n] = sum_i basis[b,i] * x[n,i]        (bf16 matmul, tiny)
      Gg [o,n] = sum_b oneh[b,o] * G[b,n]         (oneh[b,o] = r_o iff idx_o==b)
      R  [o,n] = sum_i resid'[o,i] * x[n,i]       (fp8e4m3 DoubleRow matmul)
      s'_o     = scales_o / 127
The residual term is scaled by s' <= 4e-4, so fp8 quantization error there
is negligible; the dominant basis term stays bf16. DoubleRow runs the big
R matmul at 2x bf16 rate (measured: 216ns per K=256,N=512 MM, LDW hidden).

Sharding: 8-way over tokens (1024 rows/core), transposed output layout
yT[o,n] so scales/bias are per-partition. Host pre-swizzles every operand
into its exact SBUF tile layout as a separate contiguous dram block
(contiguous transfers measured ~378GB/s vs ~120GB/s strided), and
transposes yT back. The G feed streams on two DMA rings in parallel with
the build's n-halves interleaved; dummy matmuls during the DMA ramp keep
the PE HAM clock-gate warm.
"""

import numpy as np
import ml_dtypes

import concourse.bass as bass
import concourse.mybir as mybir
import concourse.tile as tile
from concourse.bass_utils import run_bass_kernel_spmd

# problem shape (hardcoded per harness contract)
B, S, D_IN, D_OUT, BASIS = 4, 2048, 4096, 4096, 256
N_CORES = 8
N_SH = (B * S) // N_CORES           # 1024 token rows per core

P = 128
KC = D_IN // P                      # 32 contraction chunks of 128
OC = D_OUT // P                     # 32 output chunks of 128
NS = N_SH // 512                    # 2 n-slices of 512
NH = 2                              # n-halves for the G build
NG = OC // 4                        # rs8 DMA'd in groups of 4 oc-chunks

# graduated chunking of the k axis for the G/x feeds
CHUNKS = [(0, 2), (2, 4), (4, 8), (8, 16), (16, 24), (24, 32)]
X8_CHUNKS = [(0, 8), (8, 16), (16, 24), (24, 32)]

F32 = mybir.dt.float32
BF16 = mybir.dt.bfloat16
FP8 = mybir.dt.float8e4
NPF8 = ml_dtypes.float8_e4m3
NPBF = ml_dtypes.bfloat16
DR = mybir.MatmulPerfMode.DoubleRow

_WAIT_LIMIT = 1


def _split_sync_waits(nc):
    """walrus in this container rejects instructions with more than one
    embedded sync-wait command; hoist the excess onto same-engine NoOps."""
    ctr = 0
    for f in nc.m.functions:
        for bb in f.blocks:
            new = []
            changed = False
            for inst in bb.instructions:
                si = inst.sync_info
                if si is not None and si.on_wait and len(si.on_wait) > _WAIT_LIMIT:
                    waits = list(si.on_wait)
                    excess, keep = waits[:-_WAIT_LIMIT], waits[-_WAIT_LIMIT:]
                    for i in range(0, len(excess), _WAIT_LIMIT):
                        ctr += 1
                        new.append(mybir.InstNoOp(
                            name=f"I-waitsplit-{ctr}",
                            engine=inst.engine,
                            ins=[], outs=[],
                            sync_info=mybir.SyncInfo(
                                on_wait=excess[i:i + _WAIT_LIMIT], on_update=[]),
                        ))
                    si.on_wait = keep
                    changed = True
                new.append(inst)
            if changed:
                bb.instructions = new


def _build_program():
    nc = bass.Bass()
    Alu = mybir.AluOpType
    Act = mybir.ActivationFunctionType

    xt8_d = [nc.dram_tensor(f"xt8_{j}", [P, k1 - k0, N_SH], FP8,
                            kind="ExternalInput")
             for j, (k0, k1) in enumerate(X8_CHUNKS)]
    xtb_d = [[nc.dram_tensor(f"xtb{h}_{j}", [P, k1 - k0, 512], BF16,
                             kind="ExternalInput")
              for j, (k0, k1) in enumerate(CHUNKS)] for h in range(NH)]
    bast_d = [nc.dram_tensor(f"bast_{j}", [P, k1 - k0, BASIS], BF16,
                             kind="ExternalInput")
              for j, (k0, k1) in enumerate(CHUNKS)]
    oneh_d = nc.dram_tensor("oneh", [P, 2, D_OUT], BF16, kind="ExternalInput")
    rs8_d = nc.dram_tensor("rs8", [NG, P, KC, 4 * P], FP8,
                           kind="ExternalInput")
    scol_d = nc.dram_tensor("scol", [P, OC], F32, kind="ExternalInput")
    bcol_d = nc.dram_tensor("bcol", [P, OC], F32, kind="ExternalInput")
    yt_d = nc.dram_tensor("yt", [D_OUT, N_SH], BF16, kind="ExternalOutput")

    with tile.TileContext(nc) as tc:
        with (
            tc.tile_pool(name="const", bufs=1) as cpool,
            tc.tile_pool(name="rs", bufs=2) as rspool,
            tc.tile_pool(name="y", bufs=4) as ypool,
            tc.tile_pool(name="psr", bufs=4, space="PSUM") as psr,
            tc.tile_pool(name="psy", bufs=4, space="PSUM") as psy,
        ):
            # ---- PE warm-up during the DMA ramp ---------------------
            # dummy matmuls on a memset row keep the HAM activity window
            # busy so real matmuls start at 2.4GHz; memset on the vector
            # engine, which has no DMA duties.
            warm = cpool.tile([1, 64], BF16, name="warm")
            nc.vector.memset(warm[:], 1.0)
            pw = psy.tile([1, 64], F32, tag="y", name="warmps")
            for _ in range(60):
                nc.tensor.matmul(pw[:], lhsT=warm[:, 0:1], rhs=warm[:],
                                 start=True, stop=True)

            # ---- input feeds ----------------------------------------
            # ring sync:   xtb[0] chunks, then xt8 chunks
            # ring scalar: scol/bcol, xtb[1] chunks, oneh, later y-out
            # ring gpsimd: bast chunks, then rs8 groups
            scol = cpool.tile([P, OC], F32, name="scol")
            nc.scalar.dma_start(scol[:], scol_d[:])
            bcol = cpool.tile([P, OC], F32, name="bcol")
            nc.scalar.dma_start(bcol[:], bcol_d[:])

            bast_c = []
            xtb_c = [[], []]
            for j, (k0, k1) in enumerate(CHUNKS):
                bt = cpool.tile([P, k1 - k0, BASIS], BF16, name=f"bast{j}")
                nc.gpsimd.dma_start(bt[:], bast_d[j][:])
                bast_c.append(bt)
                x0 = cpool.tile([P, k1 - k0, 512], BF16, name=f"xtb0_{j}")
                nc.sync.dma_start(x0[:], xtb_d[0][j][:])
                xtb_c[0].append(x0)
                x1 = cpool.tile([P, k1 - k0, 512], BF16, name=f"xtb1_{j}")
                nc.scalar.dma_start(x1[:], xtb_d[1][j][:])
                xtb_c[1].append(x1)
            xt8_c = []
            for j, (k0, k1) in enumerate(X8_CHUNKS):
                xt = cpool.tile([P, k1 - k0, N_SH], FP8, name=f"xt8_{j}")
                nc.sync.dma_start(xt[:], xt8_d[j][:])
                xt8_c.append(xt)
            oneh = cpool.tile([P, 2, D_OUT], BF16, name="oneh")
            nc.scalar.dma_start(oneh[:], oneh_d[:])

            # ---- G build: GT[b, n] = basisT.T @ xT ------------------
            # chunk-major with both n-halves interleaved: consumes the
            # sync and scalar rings in parallel, 4 open psum groups.
            gt = cpool.tile([P, 2, N_SH], BF16, name="gt")
            ps_g = [[psy.tile([P, 512], F32, tag="y", name=f"g{h}_{bh}")
                     for bh in range(2)] for h in range(NH)]
            for j, (k0, k1) in enumerate(CHUNKS):
                for h in range(NH):
                    for bh in range(2):
                        for k in range(k0, k1):
                            nc.tensor.matmul(
                                ps_g[h][bh][:],
                                lhsT=bast_c[j][:, k - k0, bh * P:(bh + 1) * P],
                                rhs=xtb_c[h][j][:, k - k0, :],
                                start=(k == 0), stop=(k == KC - 1))
            for h in range(NH):
                for bh in range(2):
                    nc.scalar.copy(gt[:, bh, h * 512:(h + 1) * 512],
                                   ps_g[h][bh][:])

            # ---- main loop: R (fp8 DR) + gather + evac per oc -------
            def x8_slice(kk, ns):
                for j, (k0, k1) in enumerate(X8_CHUNKS):
                    if 2 * kk >= k0 and 2 * kk < k1:
                        return xt8_c[j][:, 2 * kk - k0:2 * kk - k0 + 2,
                                        ns * 512:(ns + 1) * 512]
                raise AssertionError

            rs_t = None
            for oc in range(OC):
                g, j = divmod(oc, 4)
                if j == 0:
                    rs_t = rspool.tile([P, KC, 4 * P], FP8, tag="rs",
                                       name=f"rs{g}")
                    nc.gpsimd.dma_start(rs_t[:], rs8_d[g])
                rs = rs_t[:, :, j * P:(j + 1) * P]
                ps_r = []
                for ns in range(NS):
                    pr = psr.tile([P, 512], F32, tag="r", name=f"r{oc}_{ns}")
                    for kk in range(KC // 2):
                        nc.tensor.matmul(
                            pr[:], lhsT=rs[:, 2 * kk:2 * kk + 2, :],
                            rhs=x8_slice(kk, ns),
                            start=(kk == 0), stop=(kk == KC // 2 - 1),
                            perf_mode=DR)
                    ps_r.append(pr)
                y_b = ypool.tile([P, N_SH], BF16, tag="yb", name=f"yb{oc}")
                for ns in range(NS):
                    py = psy.tile([P, 512], F32, tag="y", name=f"y{oc}_{ns}")
                    for bh in range(2):
                        nc.tensor.matmul(
                            py[:], lhsT=oneh[:, bh, oc * P:(oc + 1) * P],
                            rhs=gt[:, bh, ns * 512:(ns + 1) * 512],
                            start=(bh == 0), stop=(bh == 1))
                    # y_g = Gg + bias  (ACT, PSUM->SBUF bf16)
                    y_g = ypool.tile([P, 512], BF16, tag="yg", name=f"yg{oc}_{ns}")
                    nc.scalar.activation(y_g[:], py[:], Act.Identity,
                                         bias=bcol[:, oc:oc + 1], scale=1.0)
                    # y = R*s' + y_g  (DVE)
                    nc.vector.scalar_tensor_tensor(
                        y_b[:, ns * 512:(ns + 1) * 512], ps_r[ns][:],
                        scol[:, oc:oc + 1], y_g[:],
                        op0=Alu.mult, op1=Alu.add)
                nc.scalar.dma_start(yt_d[oc * P:(oc + 1) * P, :], y_b[:])

    _split_sync_waits(nc)
    return nc


_program_cache = {}


def _get_program():
    if "nc" not in _program_cache:
        _program_cache["nc"] = _build_program()
    return _program_cache["nc"]


def kernel(x, codes, basis_table, residual_q, residual_scales, bias):
    x = np.asarray(x, dtype=np.float32)
    codes = np.asarray(codes, dtype=np.int32)
    basis_table = np.asarray(basis_table, dtype=np.float32)
    residual_q = np.asarray(residual_q, dtype=np.int32)
    residual_scales = np.asarray(residual_scales, dtype=np.float32)
    bias = np.asarray(bias, dtype=np.float32)

    x2 = x.reshape(B * S, D_IN)

    # shared (replicated) operands ------------------------------------
    r8 = (residual_q.astype(np.float32) - 128.0).astype(NPF8)
    rt = np.ascontiguousarray(r8.T)                      # [i, o]
    rs8 = np.ascontiguousarray(
        rt.reshape(KC, P, NG, 4 * P).transpose(2, 1, 0, 3))
    # [g, p, k, 4*128 o] — 4 consecutive oc chunks per contiguous group

    bt3 = basis_table.T.astype(NPBF).reshape(KC, P, BASIS)   # [k, p, b]
    bast = {f"bast_{j}": np.ascontiguousarray(
                bt3[k0:k1].transpose(1, 0, 2))
            for j, (k0, k1) in enumerate(CHUNKS)}

    idx = codes & 0xFF
    r = ((codes >> 8) & 0xFFFF).astype(np.float32) / 65535.0
    z = np.zeros((BASIS, D_OUT), np.float32)
    z[idx, np.arange(D_OUT)] = r
    oneh = np.ascontiguousarray(
        z.astype(NPBF).reshape(2, P, D_OUT).transpose(1, 0, 2))  # [p, bh, o]

    scol = np.ascontiguousarray(
        (residual_scales / 127.0).reshape(OC, P).T).astype(np.float32)
    bcol = np.ascontiguousarray(bias.reshape(OC, P).T).astype(np.float32)

    in_maps = []
    for core in range(N_CORES):
        xT = np.ascontiguousarray(
            x2[core * N_SH:(core + 1) * N_SH].T)         # [i, n]
        x83 = xT.astype(NPF8).reshape(KC, P, N_SH)       # [k, p, n]
        xb3 = xT.astype(NPBF).reshape(KC, P, N_SH)
        m = {"oneh": oneh, "rs8": rs8, "scol": scol, "bcol": bcol, **bast}
        for j, (k0, k1) in enumerate(X8_CHUNKS):
            m[f"xt8_{j}"] = np.ascontiguousarray(
                x83[k0:k1].transpose(1, 0, 2))
        for j, (k0, k1) in enumerate(CHUNKS):
            for h in range(NH):
                m[f"xtb{h}_{j}"] = np.ascontiguousarray(
                    xb3[k0:k1, :, h * 512:(h + 1) * 512].transpose(1, 0, 2))
        in_maps.append(m)

    nc = _get_program()
    res = run_bass_kernel_spmd(nc, in_maps, core_ids=list(range(N_CORES)))

    y = np.empty((B * S, D_OUT), dtype=np.float32)
    for core in range(N_CORES):
        y[core * N_SH:(core + 1) * N_SH, :] = \
            res.results[core]["yt"].T.astype(np.float32)
    return y.reshape(B, S, D_OUT)
